# revision 2
# baseline (speedup 1.0000x reference)
"""AttentionBlock (GroupNorm + 8-head attention + proj + residual) for
Trainium2, data-parallel over batch across 8 NeuronCores. v2.

Reference computation (per batch b):
  h   = GroupNorm(x)                    # 32 groups, eps=1e-5, affine
  qkv = w_qkv @ h + b_qkv               # 1x1 conv == channel matmul
  per head (8 heads, hd=64):
    S    = q^T k * hd^-0.5              # [HW, HW]
    A    = softmax(S, axis=-1)
    h'   = v @ A^T                      # [hd, HW]
  out = w_proj @ h' + b_proj + x

v2 changes over the original baseline:
  - softmax normalization: DVE reciprocal on the denominator rows (exact,
    f32r-rounded output), broadcast across partitions with K=1 f32r
    matmuls at full PE rate (walrus requires f32r matmul outputs at base
    partition 0, so each head's normalizer gets its own full-height PSUM
    tile). Replaces the old Ln -> fp32-matmul -> Exp chain.
  - fine-grained software pipeline across the whole core: batch 1's
    GroupNorm/QKV/V chunks are interleaved between batch 0's attention
    j-tiles (and batch 0's projection inside batch 1's attention) so PE
    always has independent work while ACT chews softmax exps.
  - AV accumulation is emitted a few j-tiles behind S/exp, and each
    head-pair's normalization is deferred into the next head-pair's
    S/exp stream, hiding the reciprocal/broadcast tail.
  - big memsets moved to the (otherwise idle) GpSimd engine.
  - DMA order: small constants first (GroupNorm needs them immediately),
    then x, then weights.
"""

import collections

import numpy as np

import concourse.bass as bass
import concourse.tile as tile
from concourse import mybir
from concourse.bass_utils import run_bass_kernel_spmd

F32 = mybir.dt.float32
F32R = mybir.dt.float32r
BF16 = mybir.dt.bfloat16
AF = mybir.ActivationFunctionType
ALU = mybir.AluOpType

N_CORES = 8
B, C, H, W = 16, 512, 32, 32
HW = H * W            # 1024
NH, HD = 8, 64
GROUPS = 32
GS = C // GROUPS      # 16 channels per group
EPS = 1e-5
BPC = B // N_CORES    # 2 batches per core
CT = C // 128         # 4 channel tiles
JT = HW // 128        # 8 spatial tiles (attention j)
NSL = HW // 512       # 2 moving-dim slices of 512
NPAIR = NH // 2       # 4 head pairs
SCALE = HD ** -0.5
AV_LAG = 3            # j-tiles of S/exp emitted ahead of their AV matmuls


def _split_multi_waits(nc):
    """walrus's per-instruction sync-wait slots are limited (LDWEIGHTS and
    DMA DIRECT2D reject >1). Move excess waits onto a preceding NoOp on the
    same engine — the NX sequencer processes waits in stream order, so the
    semantics are unchanged."""
    n_split = 0
    for f in nc.m.functions:
        for bb in f.blocks:
            out = []
            for inst in bb.instructions:
                si = inst.sync_info
                if si is not None and si.on_wait and len(si.on_wait) > 1:
                    waits = list(si.on_wait)
                    evsem_ok = inst.engine in (
                        mybir.EngineType.PE, mybir.EngineType.SP
                    )
                    for w in waits[:-1]:
                        if evsem_ok:
                            carrier = mybir.InstEventSemaphore(
                                name=nc.get_next_instruction_name()
                            )
                        else:
                            # DVE/ACT/Pool: EVSEM mis-encodes ("ISA wrong
                            # length"); a bare Drain carries one wait and
                            # these engines drain after every op anyway
                            carrier = mybir.InstDrain(
                                name=nc.get_next_instruction_name()
                            )
                        carrier.engine = inst.engine
                        carrier.debug = inst.debug
                        carrier.sync_info = mybir.SyncInfo(
                            on_wait=[w], on_update=[]
                        )
                        out.append(carrier)
                        n_split += 1
                    si.on_wait = waits[-1:]
                    inst.sync_info = si
                out.append(inst)
            bb.instructions[:] = out
    return n_split


def build_nc(split_waits=True):
    nc = bass.Bass()
    x_in = nc.declare_dram_parameter("x_local", [BPC, C, HW], F32, isOutput=False)
    wqkvT = nc.declare_dram_parameter("w_qkvT", [C, 3 * C], F32, isOutput=False)
    wprojT = nc.declare_dram_parameter("w_projT", [C, C], F32, isOutput=False)
    # packed per-channel constants: [128, 28] = bq|bk|beff|gamma|beta (CT
    # cols each) | gn_ind (8 cols); one DMA instead of six
    con_d = nc.declare_dram_parameter("consts", [128, 5 * CT + GROUPS // CT], F32,
                                      isOutput=False)
    rep_d = nc.declare_dram_parameter("gn_rep", [GROUPS // CT, 128], F32, isOutput=False)
    out_d = nc.declare_dram_parameter("out_local", [BPC, C, HW], F32, isOutput=True)

    with tile.TileContext(nc) as tc:
        with (
            tc.tile_pool(name="wpool", bufs=1) as wpool,
            tc.tile_pool(name="cpool", bufs=1) as cpool,
            tc.tile_pool(name="hpool", bufs=2) as hpool,
            tc.tile_pool(name="qkpool", bufs=2) as qkpool,
            tc.tile_pool(name="vhpool", bufs=2) as vhpool,
            tc.tile_pool(name="epool", bufs=5) as epool,
            tc.tile_pool(name="spool", bufs=4) as spool,
            tc.tile_pool(name="rpool", bufs=2) as rpool,
            tc.tile_pool(name="opool", bufs=3) as opool,
            tc.tile_pool(name="ps2", bufs=2, space="PSUM") as ps2,
            tc.tile_pool(name="psx", bufs=2, space="PSUM") as psx,
            tc.tile_pool(name="psav", bufs=1, space="PSUM") as psav,
        ):
            # ---------- small constants first (GroupNorm blocks on these) ----------
            con_sb = cpool.tile([128, 5 * CT + GROUPS // CT], F32, tag="con")
            nc.sync.dma_start(out=con_sb, in_=con_d.ap())
            bq_sb = con_sb[:, 0:CT]
            bk_sb = con_sb[:, CT:2 * CT]
            beff_sb = con_sb[:, 2 * CT:3 * CT]
            gam_sb = con_sb[:, 3 * CT:4 * CT]
            bet_sb = con_sb[:, 4 * CT:5 * CT]
            ind16 = con_sb[:, 5 * CT:5 * CT + GROUPS // CT]
            rep_sb = cpool.tile([GROUPS // CT, 128], F32, tag="rep")
            nc.sync.dma_start(out=rep_sb, in_=rep_d.ap())

            eps_sb = cpool.tile([128, 1], F32, tag="eps")
            nc.vector.memset(eps_sb, EPS)
            # f32r all-ones stationary for the K=1 normalizer broadcasts
            ones_f = cpool.tile([128, 128], F32, tag="ones_f")
            nc.vector.memset(ones_f, 1.0)
            onesr = cpool.tile([128, 128], F32R, tag="onesr")
            with nc.allow_low_precision(reason="f32r rounding of exact 1.0"):
                nc.vector.tensor_copy(onesr, ones_f)

            # ---------- x(0), then q/k weights, then x(1), then v/proj ----------
            vhA_t = {}
            vhB_t = {}
            xl = {}
            xl[0] = hpool.tile([128, CT, HW], F32, tag="xl", name="xl0")
            for kt in range(CT):
                nc.gpsimd.dma_start(
                    out=xl[0][:, kt, :], in_=x_in[0, kt * 128:(kt + 1) * 128, :]
                )

            # AV stationary tiles for both batches, constant columns set once
            # on the (otherwise idle) GpSimd engine. Even heads ("A"): v in
            # cols 0-63, ones col 64 -> AV psum rows 0-63 = v@E, row 64 =
            # softmax denominator. Odd heads ("B"): v in cols 64-127
            # (lane-aligned with partitions 64-127), ones col 32, zeros
            # elsewhere -> denominator on row 32.
            for b in range(BPC):
                vhA_t[b] = vhpool.tile([128, JT, NPAIR, 65], BF16, tag="vhA",
                                       name=f"vhA{b}")
                vhB_t[b] = vhpool.tile([128, JT, NPAIR, 128], BF16, tag="vhB",
                                       name=f"vhB{b}")
                nc.gpsimd.memset(vhA_t[b][:, :, :, 64:65], 1.0)
                nc.gpsimd.memset(vhB_t[b][:, :, :, 0:64], 0.0)
                nc.gpsimd.memset(vhB_t[b][:, :, :, 32:33], 1.0)

            wq_sb = wpool.tile([128, CT, C], BF16, tag="wq")
            wk_sb = wpool.tile([128, CT, C], BF16, tag="wk")
            wv_sb = wpool.tile([128, CT, C], BF16, tag="wv")
            wp_sb = wpool.tile([128, CT, C], BF16, tag="wp")

            def load_w(w_sb, w_src, wi):
                # gpsimd (SWDGE) DMAs cast in flight: fp32 HBM -> bf16 SBUF
                nc.gpsimd.dma_start(
                    out=w_sb, in_=w_src.rearrange("(kt p) o -> p kt o", p=128)
                )

            load_w(wq_sb, wqkvT[:, 0:C], 0)
            load_w(wk_sb, wqkvT[:, C:2 * C], 1)

            xl[1] = hpool.tile([128, CT, HW], F32, tag="xl", name="xl1")
            for kt in range(CT):
                nc.gpsimd.dma_start(
                    out=xl[1][:, kt, :], in_=x_in[1, kt * 128:(kt + 1) * 128, :]
                )

            load_w(wv_sb, wqkvT[:, 2 * C:3 * C], 2)
            load_w(wp_sb, wprojT[:, :], 3)

            h_t = {}
            ho_t = {}
            q_t = {}
            k_t = {}

            def gen_gn_pipelined(b):
                """GroupNorm for batch b, one chunk per channel tile —
                longer total work than the batched variant but much lower
                latency to the first normalized tile; used for batch 0
                where GroupNorm heads the critical path."""
                h_t[b] = hpool.tile([128, CT, HW], BF16, tag="h", name=f"h{b}")
                ho_t[b] = hpool.tile([128, CT, HW], BF16, tag="ho", name=f"ho{b}")
                xl_t = xl[b]
                for kt in range(CT):
                    st = spool.tile([128, 2, 6], F32, tag="bnst")
                    for s in range(2):
                        nc.vector.bn_stats(
                            out=st[:, s, :], in_=xl_t[:, kt, s * 512:(s + 1) * 512]
                        )
                    s3 = spool.tile([128, 3], F32, tag="s3k")
                    nc.vector.bn_aggr(out=s3[:, 0:2], in_=st)
                    nc.vector.tensor_mul(s3[:, 2:3], s3[:, 0:1], s3[:, 0:1])
                    gps = psx.tile([128, 512], F32, tag="psx", name="gps")
                    nc.tensor.matmul(
                        gps[0:8, 0:3], lhsT=ind16, rhs=s3, start=True, stop=True
                    )
                    g3 = spool.tile([8, 3], F32, tag="g3k")
                    nc.vector.tensor_copy(g3, gps[0:8, 0:3])
                    g2 = spool.tile([8, 2], F32, tag="g2k")
                    nc.vector.tensor_copy(g2[:, 0:1], g3[:, 0:1])
                    vg = spool.tile([8, 2], F32, tag="vgk")
                    nc.vector.tensor_add(vg[:, 0:1], g3[:, 1:2], g3[:, 2:3])
                    nc.vector.tensor_mul(vg[:, 1:2], g3[:, 0:1], g3[:, 0:1])
                    nc.vector.tensor_sub(vg[:, 0:1], vg[:, 0:1], vg[:, 1:2])
                    nc.scalar.activation(
                        out=vg[:, 1:2], in_=vg[:, 0:1], func=AF.Ln,
                        bias=eps_sb[0:8, :], scale=1.0,
                    )
                    nc.scalar.activation(
                        out=g2[:, 1:2], in_=vg[:, 1:2], func=AF.Exp,
                        scale=-0.5,
                    )
                    bcg = psx.tile([128, 512], F32, tag="psx", name="bcg")
                    nc.tensor.matmul(
                        bcg[0:128, 0:2], lhsT=rep_sb, rhs=g2, start=True, stop=True
                    )
                    ab = spool.tile([128, 3], F32, tag="abk")
                    nc.vector.tensor_mul(ab[:, 0:1], bcg[:, 1:2], gam_sb[:, kt:kt + 1])
                    nc.vector.tensor_mul(ab[:, 2:3], bcg[:, 0:1], ab[:, 0:1])
                    nc.vector.tensor_sub(ab[:, 1:2], bet_sb[:, kt:kt + 1], ab[:, 2:3])
                    nc.vector.tensor_scalar(
                        out=h_t[b][:, kt, :], in0=xl_t[:, kt, :],
                        scalar1=ab[:, 0:1], scalar2=ab[:, 1:2],
                        op0=ALU.mult, op1=ALU.add,
                    )
                    yield

            def gen_gn(b):
                """GroupNorm for batch b, batched across the 4 channel
                tiles: one stats matmul, one Ln/Exp pair, one broadcast."""
                h_t[b] = hpool.tile([128, CT, HW], BF16, tag="h", name=f"h{b}")
                ho_t[b] = hpool.tile([128, CT, HW], BF16, tag="ho", name=f"ho{b}")
                xl_t = xl[b]
                s3 = spool.tile([128, CT, 3], F32, tag="s3")
                for kt in range(CT):
                    st = spool.tile([128, 2, 6], F32, tag="bnst",
                                    name=f"st{kt}")
                    for s in range(2):
                        nc.vector.bn_stats(
                            out=st[:, s, :], in_=xl_t[:, kt, s * 512:(s + 1) * 512]
                        )
                    nc.vector.bn_aggr(out=s3[:, kt, 0:2], in_=st)
                    if kt % 2 == 1:
                        yield
                nc.vector.tensor_mul(s3[:, :, 2:3], s3[:, :, 0:1], s3[:, :, 0:1])
                # per-group aggregation: [8, kt, 3] = (mu_g, E var_p, E mu_p^2)
                gps = psx.tile([128, 512], F32, tag="psx", name="gps")
                nc.tensor.matmul(
                    gps[0:8, 0:3 * CT],
                    lhsT=ind16, rhs=s3.rearrange("p m t -> p (m t)"),
                    start=True, stop=True,
                )
                g3 = spool.tile([8, CT, 3], F32, tag="g3")
                nc.vector.tensor_copy(
                    g3, gps[0:8, 0:3 * CT].rearrange("p (m t) -> p m t", t=3)
                )
                g2 = spool.tile([8, CT, 2], F32, tag="g2")
                nc.vector.tensor_copy(g2[:, :, 0:1], g3[:, :, 0:1])
                vg = spool.tile([8, CT, 2], F32, tag="vg")
                nc.vector.tensor_add(vg[:, :, 0:1], g3[:, :, 1:2], g3[:, :, 2:3])
                nc.vector.tensor_mul(vg[:, :, 1:2], g3[:, :, 0:1], g3[:, :, 0:1])
                nc.vector.tensor_sub(vg[:, :, 0:1], vg[:, :, 0:1], vg[:, :, 1:2])
                # rstd = exp(-0.5*ln(var+eps)): keeps every activation in
                # the natural_log_exp table set (no ACT table switches)
                nc.scalar.activation(
                    out=vg[:, :, 1:2], in_=vg[:, :, 0:1], func=AF.Ln,
                    bias=eps_sb[0:8, :], scale=1.0,
                )
                nc.scalar.activation(
                    out=g2[:, :, 1:2], in_=vg[:, :, 1:2], func=AF.Exp,
                    scale=-0.5,
                )
                # broadcast (mu_g, rstd_g) per kt to all 128 channel partitions
                bcg = psx.tile([128, 512], F32, tag="psx", name="bcg")
                nc.tensor.matmul(
                    bcg[0:128, 0:2 * CT],
                    lhsT=rep_sb, rhs=g2.rearrange("p m t -> p (m t)"),
                    start=True, stop=True,
                )
                bsc = spool.tile([128, CT, 3], F32, tag="absc")
                nc.vector.tensor_copy(
                    bsc[:, :, 0:2],
                    bcg[0:128, 0:2 * CT].rearrange("p (m t) -> p m t", t=2),
                )
                ab = spool.tile([128, CT, 2], F32, tag="ab")
                gam3 = gam_sb.rearrange("p (m o) -> p m o", o=1)
                bet3 = bet_sb.rearrange("p (m o) -> p m o", o=1)
                nc.vector.tensor_mul(ab[:, :, 0:1], bsc[:, :, 1:2], gam3)
                nc.vector.tensor_mul(bsc[:, :, 2:3], bsc[:, :, 0:1], ab[:, :, 0:1])
                nc.vector.tensor_sub(ab[:, :, 1:2], bet3, bsc[:, :, 2:3])
                yield
                for kt in range(CT):
                    nc.vector.tensor_scalar(
                        out=h_t[b][:, kt, :], in0=xl_t[:, kt, :],
                        scalar1=ab[:, kt, 0:1], scalar2=ab[:, kt, 1:2],
                        op0=ALU.mult, op1=ALU.add,
                    )
                    if kt % 2 == 1:
                        yield

            def gen_qk(b, ms=None):
                """q/k projections for batch b; one chunk per (m, q|k)."""
                if ms is None or ms[0] == 0:
                    q_t[b] = qkpool.tile([128, CT, HW], BF16, tag="q", name=f"q{b}")
                    k_t[b] = qkpool.tile([128, CT, HW], BF16, tag="k", name=f"k{b}")
                for m in (ms if ms is not None else range(CT)):
                    for w_sb, b_sb, dst in (
                        (wq_sb, bq_sb, q_t[b]), (wk_sb, bk_sb, k_t[b]),
                    ):
                        for isl in range(NSL):
                            pq = psx.tile([128, 512], F32, tag="psx", name="pq")
                            for kt in range(CT):
                                nc.tensor.matmul(
                                    pq[:, :],
                                    lhsT=w_sb[:, kt, m * 128:(m + 1) * 128],
                                    rhs=h_t[b][:, kt, isl * 512:(isl + 1) * 512],
                                    start=(kt == 0), stop=(kt == CT - 1),
                                )
                            nc.vector.tensor_scalar(
                                out=dst[:, m, isl * 512:(isl + 1) * 512],
                                in0=pq[:, :],
                                scalar1=b_sb[:, m:m + 1], scalar2=None, op0=ALU.add,
                            )
                            yield

            def gen_v(b):
                """v (transposed layout) for batch b; one chunk per j-tile."""
                for mj in range(JT):
                    pv = psx.tile([128, 512], F32, tag="psx", name="pv")
                    for kt in range(CT):
                        nc.tensor.matmul(
                            pv[:, :],
                            lhsT=h_t[b][:, kt, mj * 128:(mj + 1) * 128],
                            rhs=wv_sb[:, kt, :],
                            start=(kt == 0), stop=(kt == CT - 1),
                        )
                    pv_h = pv[:, :].rearrange(
                        "p (hp a d) -> p hp a d", hp=NPAIR, a=2
                    )
                    nc.vector.tensor_copy(vhA_t[b][:, mj, :, 0:64], pv_h[:, :, 0, :])
                    nc.vector.tensor_copy(vhB_t[b][:, mj, :, 64:128], pv_h[:, :, 1, :])
                    yield

            def gen_proj(b):
                """Projection + residual + store for batch b; chunk per m."""
                for m in range(CT):
                    for isl in range(NSL):
                        po = psx.tile([128, 512], F32, tag="psx", name="po")
                        for kt in range(CT):
                            nc.tensor.matmul(
                                po[:, :],
                                lhsT=wp_sb[:, kt, m * 128:(m + 1) * 128],
                                rhs=ho_t[b][:, kt, isl * 512:(isl + 1) * 512],
                                start=(kt == 0), stop=(kt == CT - 1),
                            )
                        ot = opool.tile([128, 512], F32, tag="ot")
                        nc.vector.scalar_tensor_tensor(
                            out=ot, in0=po[:, :], scalar=beff_sb[:, m:m + 1],
                            in1=xl[b][:, m, isl * 512:(isl + 1) * 512],
                            op0=ALU.add, op1=ALU.add,
                        )
                        nc.sync.dma_start(
                            out=out_d[b, m * 128:(m + 1) * 128,
                                      isl * 512:(isl + 1) * 512],
                            in_=ot,
                        )
                        yield

            # deferred normalization closure for the previous (hp, i-half)
            pending_norm = [None]

            def make_norm(b, hp, ih, avA, avB):
                def emit_norm():
                    # exact DVE reciprocal on the denominator rows
                    # (f32r-rounded), broadcast across partitions with K=1
                    # f32r matmuls (full PE rate; f32r matmul outputs must
                    # start at partition 0, so each head gets a full-height
                    # broadcast), then multiply into h_t. Both heads share
                    # one 2-bank PSUM tile (disjoint 512-col halves).
                    rr = rpool.tile([128, 512], F32R, tag="rr")
                    with nc.allow_low_precision(
                        reason="f32r rounding of softmax normalizer (~1e-4)"
                    ):
                        nc.vector.reciprocal(rr[64:65, :], avA[64:65, :])
                        nc.vector.reciprocal(rr[32:33, :], avB[32:33, :])
                    bcA = psx.tile([128, 512], F32, tag="psx", name="bcA")
                    nc.tensor.matmul(
                        bcA[:, :], lhsT=onesr[64:65, :], rhs=rr[64:65, :],
                        start=True, stop=True,
                    )
                    bcB = psx.tile([128, 512], F32, tag="psx", name="bcB")
                    nc.tensor.matmul(
                        bcB[:, :], lhsT=onesr[32:33, :], rhs=rr[32:33, :],
                        start=True, stop=True,
                    )
                    # DVE ops may read only one PSUM operand: land the
                    # broadcast in SBUF first
                    bsb = rpool.tile([128, 1024], F32, tag="bsb")
                    nc.vector.tensor_copy(bsb[:, 0:512], bcA[:, :])
                    nc.vector.tensor_copy(bsb[:, 512:1024], bcB[:, :])
                    sl = slice(ih * 512, (ih + 1) * 512)
                    nc.vector.tensor_mul(
                        ho_t[b][0:64, hp, sl], avA[0:64, :], bsb[0:64, 0:512],
                    )
                    nc.vector.tensor_mul(
                        ho_t[b][64:128, hp, sl], avB[64:128, :], bsb[64:128, 512:1024],
                    )
                return emit_norm

            def pull(it):
                if it is not None:
                    try:
                        next(it)
                    except StopIteration:
                        pass

            def run_attn(b, hp, ih, fillers):
                """S/exp stream for one (head-pair, i-half): AV emitted
                AV_LAG j-tiles behind, the previous chunk's normalization
                emitted after jb 1, fillers pulled every other j-tile."""
                avA = psav.tile([65, 512], F32, tag="avA")
                avB = psav.tile([128, 512], F32, tag="avB")
                pend = collections.deque()

                def emit_av(jb, e_t):
                    nc.tensor.matmul(
                        avA[:, :],
                        lhsT=vhA_t[b][:, jb, hp, :],
                        rhs=e_t[:, 0:512],
                        start=(jb == 0), stop=(jb == JT - 1),
                    )
                    nc.tensor.matmul(
                        avB[:, :],
                        lhsT=vhB_t[b][:, jb, hp, :],
                        rhs=e_t[:, 512:1024],
                        start=(jb == 0), stop=(jb == JT - 1),
                    )

                for jb in range(JT):
                    e_t = epool.tile([128, 1024], BF16, tag="e")
                    pss = ps2.tile([128, 1024], F32, tag="ps2t", name="pss")
                    # S^T[j, i-half] for both heads of the pair, side by side
                    for a in range(2):
                        base = a * 64
                        nc.tensor.matmul(
                            pss[:, a * 512:(a + 1) * 512],
                            lhsT=k_t[b][base:base + 64, hp, jb * 128:(jb + 1) * 128],
                            rhs=q_t[b][base:base + 64, hp, ih * 512:(ih + 1) * 512],
                            start=True, stop=True,
                        )
                    nc.scalar.activation(
                        out=e_t[:, :], in_=pss[:, :], func=AF.Exp, scale=SCALE,
                    )
                    pend.append((jb, e_t))
                    if jb == 1 and pending_norm[0] is not None:
                        pending_norm[0]()
                        pending_norm[0] = None
                    pull(fillers)
                    if len(pend) > AV_LAG:
                        emit_av(*pend.popleft())
                while pend:
                    emit_av(*pend.popleft())
                pending_norm[0] = make_norm(b, hp, ih, avA, avB)

            # ---------- software-pipelined schedule over the 2 batches ----------
            import itertools

            def roundrobin(*its):
                its = [iter(i) for i in its]
                while its:
                    nxt = []
                    for i in its:
                        try:
                            next(i)
                        except StopIteration:
                            continue
                        nxt.append(i)
                        yield
                    its = nxt

            for _ in gen_gn_pipelined(0):
                pass
            for _ in gen_qk(0, [0]):
                pass

            # batch 0's remaining prep (v before later qk m's — AV needs v
            # from the 4th j-tile of head-pair 0 on), then batch 1's prep,
            # all pulled one chunk per attention j-tile
            fill_b1 = itertools.chain(
                gen_v(0), gen_qk(0, [1, 2, 3]),
                gen_gn(1), gen_qk(1), gen_v(1),
            )
            for hp in range(NPAIR):
                for ih in range(NSL):
                    run_attn(0, hp, ih, fill_b1)
            for _ in fill_b1:   # drain any leftover batch-1 prep
                pass
            pending_norm[0]()
            pending_norm[0] = None
            fill_p0 = gen_proj(0)
            for hp in range(NPAIR):
                for ih in range(NSL):
                    run_attn(1, hp, ih, fill_p0)
            pending_norm[0]()
            pending_norm[0] = None
            for _ in fill_p0:
                pass
            for _ in gen_proj(1):
                pass

    if split_waits:
        _split_multi_waits(nc)
    return nc


_NC_CACHE = {}


def _get_nc():
    if "nc" not in _NC_CACHE:
        _NC_CACHE["nc"] = build_nc()
    return _NC_CACHE["nc"]


def make_in_maps(x, gn_gamma, gn_beta, w_qkv, b_qkv, w_proj, b_proj):
    f = np.float32
    x = np.ascontiguousarray(np.asarray(x, dtype=f)).reshape(B, C, HW)
    w_qkvT = np.ascontiguousarray(np.asarray(w_qkv, dtype=f).T)
    w_projT = np.ascontiguousarray(np.asarray(w_proj, dtype=f).T)
    b_qkv = np.asarray(b_qkv, dtype=f)
    b_q = np.ascontiguousarray(b_qkv[0:C])
    b_k = np.ascontiguousarray(b_qkv[C:2 * C])
    b_v = b_qkv[2 * C:3 * C]
    # softmax rows sum to 1, so v's bias passes straight through attention:
    # fold it into the projection bias.
    b_eff = np.ascontiguousarray(
        np.asarray(w_proj, dtype=f) @ b_v + np.asarray(b_proj, dtype=f)
    )
    gn_gamma = np.ascontiguousarray(np.asarray(gn_gamma, dtype=f))
    gn_beta = np.ascontiguousarray(np.asarray(gn_beta, dtype=f))
    n_gpt = GROUPS // CT   # groups per 128-channel tile
    gn_ind = np.zeros((128, n_gpt), dtype=f)
    gn_rep = np.zeros((n_gpt, 128), dtype=f)
    for g in range(n_gpt):
        gn_ind[g * GS:(g + 1) * GS, g] = 1.0 / GS
        gn_rep[g, g * GS:(g + 1) * GS] = 1.0
    consts = np.empty((128, 5 * CT + GROUPS // CT), dtype=f)
    for i, v in enumerate((b_q, b_k, b_eff, gn_gamma, gn_beta)):
        consts[:, i * CT:(i + 1) * CT] = v.reshape(CT, 128).T
    consts[:, 5 * CT:] = gn_ind
    consts = np.ascontiguousarray(consts)
    in_maps = []
    for c in range(N_CORES):
        in_maps.append({
            "x_local": np.ascontiguousarray(x[c * BPC:(c + 1) * BPC]),
            "w_qkvT": w_qkvT,
            "w_projT": w_projT,
            "consts": consts,
            "gn_rep": gn_rep,
        })
    return in_maps


def kernel(x, gn_gamma, gn_beta, w_qkv, b_qkv, w_proj, b_proj):
    nc = _get_nc()
    in_maps = make_in_maps(x, gn_gamma, gn_beta, w_qkv, b_qkv, w_proj, b_proj)
    res = run_bass_kernel_spmd(nc, in_maps, list(range(N_CORES)))
    out = np.empty((B, C, HW), dtype=np.float32)
    for c in range(N_CORES):
        out[c * BPC:(c + 1) * BPC] = res.results[c]["out_local"]
    return out.reshape(B, C, H, W)
